# revision 1
# baseline (speedup 1.0000x reference)
"""DRGCN message-passing kernel for 8 Trainium2 NeuronCores.

Strategy: shard by destination-node range (12500 nodes/core) so each core
computes its output rows fully locally (no collectives). Host preprocesses
edges into a padded, (tile, relation)-sorted slot layout and pre-gathers
source features into a streaming-friendly layout; the device does the
segment mean (one-hot scatter matmuls with an in-pass count column), the
basis-decomposed per-relation weight composition, per-relation transforms,
and the root/bias term.
"""
import numpy as np

N_NODES = 100000
IN_C = 64
OUT_C = 64
NUM_REL = 8
R2 = 2 * NUM_REL            # 16
NUM_M, NUM_N, NUM_O = 4, 2, 1
NUM_BASES = NUM_M + NUM_N * NUM_REL + NUM_O * R2  # 36
P = 128
NCORES = 8
NPC = N_NODES // NCORES     # 12500 nodes per core
NTILES = (NPC + P - 1) // P  # 98
NPAD = NTILES * P           # 12544
J = 32                      # groups per z-chunk DMA
SENTINEL = 999.0


def _build_weight_mask():
    m = np.zeros((R2, NUM_BASES), dtype=np.float32)
    m[:, :NUM_M] = 1.0
    for row_i in range(R2):
        for col_i in range(NUM_REL):
            if row_i == col_i or row_i == col_i + NUM_REL:
                c = col_i * NUM_N
                m[row_i, NUM_M + c:NUM_M + c + NUM_N] = 1.0
        for col_i in range(R2):
            if row_i == col_i:
                s = NUM_M + NUM_N * NUM_REL + col_i * NUM_O
                m[row_i, s:s + NUM_O] = 1.0
    return m


def _host_prep(x, edge_index, edge_type):
    """Returns per-core arrays + shared group structure."""
    src = np.concatenate([edge_index[0], edge_index[1]]).astype(np.int64)
    dst = np.concatenate([edge_index[1], edge_index[0]]).astype(np.int64)
    rel = np.concatenate([edge_type, edge_type + NUM_REL]).astype(np.int64)

    core = dst // NPC
    dst_local = dst - core * NPC
    tile_id = dst_local // P
    key = tile_id * R2 + rel  # run id within core, tile-major rel-inner

    per_core = []
    run_counts = np.zeros((NCORES, NTILES * R2), dtype=np.int64)
    for c in range(NCORES):
        m = core == c
        s_c, dl_c, k_c = src[m], dst_local[m], key[m]
        order = np.argsort(k_c, kind="stable")
        s_c, dl_c, k_c = s_c[order], dl_c[order], k_c[order]
        run_counts[c] = np.bincount(k_c, minlength=NTILES * R2)
        per_core.append((s_c, dl_c, k_c))

    g_run = (np.max(run_counts, axis=0) + P - 1) // P  # groups per run, shared
    G = int(g_run.sum())
    run_g0 = np.concatenate([[0], np.cumsum(g_run)])[:-1]

    x_aug = np.concatenate(
        [x.astype(np.float32), np.ones((N_NODES, 1), np.float32)], axis=1)  # [N,65]

    cores_data = []
    for c in range(NCORES):
        s_c, dl_c, k_c = per_core[c]
        cnt_c = run_counts[c]
        run_starts = np.concatenate([[0], np.cumsum(cnt_c)])[:-1]
        slot_src = np.zeros(G * P, dtype=np.int64)
        slot_dst = np.full(G * P, SENTINEL, dtype=np.float32)
        # place each run's edges at its slot offset
        nz = np.nonzero(cnt_c)[0]
        for kr in nz:
            n = cnt_c[kr]
            s0 = run_g0[kr] * P
            e0 = run_starts[kr]
            slot_src[s0:s0 + n] = s_c[e0:e0 + n]
            t = kr // R2
            slot_dst[s0:s0 + n] = (dl_c[e0:e0 + n] - t * P).astype(np.float32)
        # z stream layout [128, G*65]: row p holds concat over g of x_aug[src[g*128+p]]
        z = x_aug[slot_src].reshape(G, P, IN_C + 1).transpose(1, 0, 2)
        z = np.ascontiguousarray(z).reshape(P, G * (IN_C + 1))
        dst_cols = slot_dst.reshape(G, P).T.copy()  # [128, G]
        # padded transposed x slice for the root term
        xt = np.zeros((IN_C, NPAD), dtype=np.float32)
        xt[:, :NPC] = x[c * NPC:(c + 1) * NPC].T
        cores_data.append({"z": z, "dstc": dst_cols, "xt": xt})
    return cores_data, g_run, G


def _build_program(g_run, G, fp16=False, repeat=1, nopipe=True, v2=True):
    import concourse.tile as tile
    from concourse import bass, bacc, mybir
    from contextlib import ExitStack

    f32 = mybir.dt.float32
    agg_dt = mybir.dt.float16 if fp16 else f32
    nc = bacc.Bacc("TRN2", target_bir_lowering=False, debug=False,
                   num_devices=NCORES)
    NCHUNK = (G + J - 1) // J
    GP = NCHUNK * J  # padded group count for chunked streaming

    z_dram = nc.declare_dram_parameter("z", [P, GP * (IN_C + 1)], agg_dt, isOutput=False)
    dst_dram = nc.declare_dram_parameter("dstc", [P, GP], f32, isOutput=False)
    xt_dram = nc.declare_dram_parameter("xt", [IN_C, NPAD], f32, isOutput=False)
    wgt_dram = nc.declare_dram_parameter("wgt", [NUM_BASES, IN_C * OUT_C], f32, isOutput=False)
    mct_dram = nc.declare_dram_parameter("mct", [NUM_BASES, R2], f32, isOutput=False)
    root_dram = nc.declare_dram_parameter("root", [IN_C, OUT_C], f32, isOutput=False)
    bias_dram = nc.declare_dram_parameter("bias", [OUT_C, 1], f32, isOutput=False)
    iota_dram = nc.declare_dram_parameter("iota", [P, P], agg_dt, isOutput=False)
    ident_dram = nc.declare_dram_parameter("ident", [P, P], agg_dt, isOutput=False)
    out_dram = nc.declare_dram_parameter("out", [OUT_C, NPAD], f32, isOutput=True)

    w_scratch = nc.dram_tensor("w_scratch", [R2, IN_C, OUT_C], f32)

    with tile.TileContext(nc) as tc:
        with ExitStack() as ctx:
            const_p = ctx.enter_context(tc.tile_pool(name="const", bufs=1, space="SBUF"))
            zchunk_p = ctx.enter_context(tc.tile_pool(name="zchunk", bufs=4, space="SBUF"))
            oh_p = ctx.enter_context(tc.tile_pool(name="oh", bufs=8, space="SBUF"))
            small_p = ctx.enter_context(tc.tile_pool(name="small", bufs=8, space="SBUF"))
            aggt_p = ctx.enter_context(tc.tile_pool(name="aggt", bufs=4, space="SBUF"))
            out_p = ctx.enter_context(tc.tile_pool(name="outs", bufs=3, space="SBUF"))
            ps_agg_p = ctx.enter_context(tc.tile_pool(name="psagg", bufs=2, space="PSUM"))
            ps_t_p = ctx.enter_context(tc.tile_pool(name="pst", bufs=2, space="PSUM"))
            ps_out_p = ctx.enter_context(tc.tile_pool(name="psout", bufs=2, space="PSUM"))

            iota_t = const_p.tile([P, P], agg_dt)
            nc.sync.dma_start(out=iota_t[:], in_=iota_dram[:])
            ident_t = const_p.tile([P, P], agg_dt)
            nc.sync.dma_start(out=ident_t[:], in_=ident_dram[:])
            root_t = const_p.tile([IN_C, OUT_C], f32)
            nc.sync.dma_start(out=root_t[:], in_=root_dram[:])
            bias_t = const_p.tile([OUT_C, 1], f32)
            nc.sync.dma_start(out=bias_t[:], in_=bias_dram[:])

            # ---- weight composition: W[r] = (mask*comp @ weight_flat)[r] ----
            mct_t = const_p.tile([NUM_BASES, R2], f32)
            nc.sync.dma_start(out=mct_t[:], in_=mct_dram[:])
            wgt_t = const_p.tile([NUM_BASES, IN_C * OUT_C], f32)
            nc.sync.dma_start(out=wgt_t[:], in_=wgt_dram[:])
            w_all = const_p.tile([R2, IN_C * OUT_C], f32)
            for k in range(IN_C * OUT_C // 512):
                ps_w = ps_agg_p.tile([R2, 512], f32, space="PSUM")
                nc.tensor.matmul(out=ps_w[:], lhsT=mct_t[:],
                                 rhs=wgt_t[:, k * 512:(k + 1) * 512],
                                 start=True, stop=True)
                nc.vector.tensor_copy(out=w_all[:, k * 512:(k + 1) * 512], in_=ps_w[:])
            nc.sync.dma_start(out=w_scratch[:, :, :], in_=w_all[:])
            w_tiles = []
            for r in range(R2):
                w_r32 = const_p.tile([IN_C, OUT_C], f32, name=f"w_r32_{r}")
                nc.sync.dma_start(out=w_r32[:], in_=w_scratch[r, :, :])
                if fp16:
                    w_r = const_p.tile([IN_C, OUT_C], agg_dt, name=f"w_r{r}")
                    nc.vector.tensor_copy(out=w_r[:], in_=w_r32[:])
                else:
                    w_r = w_r32
                w_tiles.append(w_r)

            # ---- main loop ----
            CW = IN_C + 1  # 65 columns per group in the z stream
            zt = None
            dt = None
            g_cum = np.concatenate([[0], np.cumsum(g_run)])
            pending = [None]

            def make_tail(ps_agg, r, ps_out, stop_flag, fin_t):
                def tail(ps_agg=ps_agg, r=r, ps_out=ps_out,
                         stop_flag=stop_flag, fin_t=fin_t):
                    cnt_cl = small_p.tile([P, 1], f32, name="cnt_cl")
                    nc.vector.tensor_scalar(out=cnt_cl[:], in0=ps_agg[:, IN_C:CW],
                                            scalar1=1.0, scalar2=None,
                                            op0=mybir.AluOpType.max)
                    recip = small_p.tile([P, 1], f32, name="recip")
                    nc.vector.reciprocal(out=recip[:], in_=cnt_cl[:])
                    agg = aggt_p.tile([P, IN_C], agg_dt, name="agg")
                    if v2:
                        nc.scalar.activation(
                            out=agg[:], in_=ps_agg[:, 0:IN_C],
                            func=mybir.ActivationFunctionType.Identity,
                            scale=recip[:, 0:1])
                    else:
                        nc.vector.tensor_scalar(out=agg[:], in0=ps_agg[:, 0:IN_C],
                                                scalar1=recip[:, 0:1], scalar2=None,
                                                op0=mybir.AluOpType.mult)
                    ps_t = ps_t_p.tile([IN_C, P], agg_dt, space="PSUM", name="ps_t")
                    nc.tensor.transpose(out=ps_t[:], in_=agg[:], identity=ident_t[:])
                    aggT = aggt_p.tile([IN_C, P], agg_dt, name="aggT")
                    nc.scalar.activation(out=aggT[:], in_=ps_t[:],
                                         func=mybir.ActivationFunctionType.Copy)
                    nc.tensor.matmul(out=ps_out[:], lhsT=w_tiles[r], rhs=aggT[:],
                                     start=False, stop=stop_flag)
                    if fin_t is not None:
                        o_sb = out_p.tile([OUT_C, P], f32, name="o_sb")
                        nc.scalar.activation(
                            out=o_sb[:], in_=ps_out[:],
                            func=mybir.ActivationFunctionType.Identity,
                            bias=bias_t[:, 0:1])
                        nc.sync.dma_start(
                            out=out_dram[:, fin_t * P:(fin_t + 1) * P], in_=o_sb[:])
                return tail

            for rep in range(repeat):
              zt_ch = -1
              for t in range(NTILES):
                  xt_t = small_p.tile([IN_C, P], f32, name="xt_t")
                  nc.sync.dma_start(out=xt_t[:], in_=xt_dram[:, t * P:(t + 1) * P])
                  ps_out = ps_out_p.tile([OUT_C, P], f32, space="PSUM", name="ps_out")
                  runs = [r for r in range(R2) if g_run[t * R2 + r] > 0]
                  nc.tensor.matmul(out=ps_out[:], lhsT=root_t[:], rhs=xt_t[:],
                                   start=True, stop=(len(runs) == 0))
                  if not runs:
                      o_sb = out_p.tile([OUT_C, P], f32, name="o_sb")
                      nc.scalar.activation(
                          out=o_sb[:], in_=ps_out[:],
                          func=mybir.ActivationFunctionType.Identity,
                          bias=bias_t[:, 0:1])
                      nc.sync.dma_start(out=out_dram[:, t * P:(t + 1) * P],
                                        in_=o_sb[:])
                      continue
                  for ri, r in enumerate(runs):
                      kr = t * R2 + r
                      g0 = int(g_cum[kr])
                      ng = int(g_run[kr])
                      ps_agg = ps_agg_p.tile([P, CW], f32, space="PSUM", name="ps_agg")
                      for k in range(ng):
                          g = g0 + k
                          ch, gl = g // J, g % J
                          if ch != zt_ch:
                              zt = zchunk_p.tile([P, J * CW], agg_dt, name="zt")
                              nc.sync.dma_start(
                                  out=zt[:], in_=z_dram[:, ch * J * CW:(ch + 1) * J * CW])
                              dt = zchunk_p.tile([P, J], f32, name="dt")
                              nc.sync.dma_start(
                                  out=dt[:], in_=dst_dram[:, ch * J:(ch + 1) * J])
                              zt_ch = ch
                          oh = oh_p.tile([P, P], agg_dt, name="oh")
                          oh_eng = nc.gpsimd if (v2 and k % 4 == 3) else nc.vector
                          oh_eng.tensor_scalar(
                              out=oh[:], in0=iota_t[:], scalar1=dt[:, gl:gl + 1],
                              scalar2=None, op0=mybir.AluOpType.is_equal)
                          nc.tensor.matmul(out=ps_agg[:], lhsT=oh[:],
                                           rhs=zt[:, gl * CW:(gl + 1) * CW],
                                           start=(k == 0), stop=(k == ng - 1))
                      if pending[0] is not None:
                          pending[0]()
                      pending[0] = make_tail(ps_agg, r, ps_out,
                                             stop_flag=(ri == len(runs) - 1),
                                             fin_t=(t if ri == len(runs) - 1 else None))
                      if nopipe:
                          pending[0]()
                          pending[0] = None
            if pending[0] is not None:
                pending[0]()
                pending[0] = None

    nc.compile()
    return nc


def prepare(x, edge_index, edge_type, weight, comp, root, bias,
            fp16=None, repeat=1, nopipe=True, v2=True):
    import os
    if fp16 is None:
        fp16 = not bool(os.environ.get("DRGCN_F32"))
    x = np.asarray(x, dtype=np.float32)
    edge_index = np.asarray(edge_index)
    edge_type = np.asarray(edge_type)
    weight = np.asarray(weight, dtype=np.float32)
    comp = np.asarray(comp, dtype=np.float32)
    root = np.asarray(root, dtype=np.float32)
    bias = np.asarray(bias, dtype=np.float32)

    cores_data, g_run, G = _host_prep(x, edge_index, edge_type)
    nc = _build_program(g_run, G, fp16=fp16, repeat=repeat, nopipe=nopipe, v2=v2)

    mask = _build_weight_mask()
    mct = np.ascontiguousarray((mask * comp).T)          # [36, 16]
    wgt = weight.reshape(NUM_BASES, IN_C * OUT_C)
    zdt = np.float16 if fp16 else np.float32
    iota = np.tile(np.arange(P, dtype=zdt)[None, :], (P, 1))
    ident = np.eye(P, dtype=zdt)
    bias_col = bias.reshape(OUT_C, 1)

    NCHUNK = (G + J - 1) // J
    GP = NCHUNK * J
    in_maps = []
    for c in range(NCORES):
        d = cores_data[c]
        z = d["z"].astype(zdt)
        if GP != G:  # pad stream to chunk multiple
            zp = np.zeros((P, GP * (IN_C + 1)), zdt)
            zp[:, :G * (IN_C + 1)] = z
            z = zp
            dc = np.full((P, GP), SENTINEL, np.float32)
            dc[:, :G] = d["dstc"]
        else:
            dc = d["dstc"]
        in_maps.append({
            "z": z, "dstc": dc, "xt": d["xt"], "wgt": wgt, "mct": mct,
            "root": root, "bias": bias_col, "iota": iota, "ident": ident,
        })

    return nc, in_maps


def assemble(results):
    out = np.empty((N_NODES, OUT_C), dtype=np.float32)
    for c in range(NCORES):
        out[c * NPC:(c + 1) * NPC] = results[c]["out"][:, :NPC].T
    return out


def kernel(x, edge_index, edge_type, weight, comp, root, bias):
    from concourse.bass_utils import run_bass_kernel_spmd

    nc, in_maps = prepare(x, edge_index, edge_type, weight, comp, root, bias)
    res = run_bass_kernel_spmd(nc, in_maps, core_ids=list(range(NCORES)))
    return assemble(res.results)

